# revision 1
# baseline (speedup 1.0000x reference)
"""BatchMultiHeadGraphAttention kernel for TRN2 (8 NeuronCores).

Reference computation (per graph b):
  h_prime = h @ w[head]                 [n, fo] per head
  t = tanh(h_prime)
  src[n] = t @ a_src[head];  dst[n] = t @ a_dst[head]
  s[i, j] = leaky_relu(src[i] + dst[j], 0.2)
  s masked where ~(adj | I); softmax over j; out = p @ h_prime + bias

Sharding: data-parallel over batch — one graph per core (BS=8, 8 cores).

Per-core layout strategy (scores kept transposed so the second matmul
contracts over the partition axis):
  - src/dst node scores via hpT = w.T @ h per head (PE), tanh (ACT), then a
    k=64 PE matmul against [a_src|a_dst] -> (src; dst) rows directly.
  - sT[j, i] = dst_j + src_i via a k=2 PE matmul into PSUM (bf16 operands).
  - Leaky: ACT Prelu(alpha=0.2) PSUM->SBUF bf16 for 5 of 8 j-blocks; DVE
    (0.2*x then max(5*x, x)) for the other 3 — balances the two engines.
  - Exp on ACT (2 instrs/head, bf16 out); adjacency mask = bf16 DVE multiply
    with valid^T built by u8->bf16 convert (Pool) + DMA transpose.
  - out[i, o] = sum_j p[j, i] * hp[j, o]: p chunks stationary with stride-8
    column interleave so the output DMA gets 2KB-contiguous descriptors; a
    ones column in hp yields the softmax denominator Z from the same matmul;
    bias is pre-added to hp (valid because sum_j p/Z = 1).
"""

import sys

import numpy as np

try:
    import concourse.bass  # noqa: F401
except ImportError:
    sys.path.insert(0, "/opt/trn_rl_repo")

BS, N, H, FI, FO = 8, 1024, 8, 256, 64
P = 128
NB = N // P     # 8 node blocks
FC = FI // P    # 2 f_in chunks
ALPHA = 0.2


def build_bass():
    import concourse.bass as bass
    import concourse.mybir as mybir
    from concourse import bacc
    from concourse.masks import make_identity
    from concourse.tile import TileContext

    f32 = mybir.dt.float32
    bf16 = mybir.dt.bfloat16
    u8 = mybir.dt.uint8
    AF = mybir.ActivationFunctionType
    OP = mybir.AluOpType
    AX = mybir.AxisListType

    nc = bacc.Bacc(trn_type="TRN2")

    h_d = nc.dram_tensor("h", [N, FI], f32, kind="ExternalInput")
    adj_d = nc.dram_tensor("adj", [N, N], u8, kind="ExternalInput")
    w_d = nc.dram_tensor("w", [H, FI, FO], f32, kind="ExternalInput")
    asrc_d = nc.dram_tensor("a_src", [H, FO], f32, kind="ExternalInput")
    adst_d = nc.dram_tensor("a_dst", [H, FO], f32, kind="ExternalInput")
    bias_d = nc.dram_tensor("bias", [FO], f32, kind="ExternalInput")
    out_d = nc.dram_tensor("out", [H, N, FO], f32, kind="ExternalOutput")

    with TileContext(nc) as tc:
        with (
            tc.tile_pool(name="singles", bufs=1) as singles,
            tc.tile_pool(name="temps", bufs=2) as temps,
            tc.tile_pool(name="ppool", bufs=2) as ppool,
            tc.tile_pool(name="spool", bufs=2) as spool,
        ):
            # ---------- long-lived tiles ----------
            ident = singles.tile([P, P], f32)
            make_identity(nc, ident)

            hT_sb = singles.tile([P, FC, N], bf16)  # [f_local, fc, n]
            # score-matmul operands: sd2 row0 = dstrow per head, row1 = ones
            #                        os2 row0 = ones, row1 = srcrow per head
            sd2 = singles.tile([2, H, N], bf16)
            os2 = singles.tile([2, H, N], bf16)
            ones_small = singles.tile([P, FO], bf16)
            nc.vector.memset(ones_small, 1.0)
            # replicate ones into sd2 row1 / os2 row0 (DMA; data rows DMA'd later)
            nc.sync.dma_start(out=sd2[1:2], in_=ones_small)
            nc.sync.dma_start(out=os2[0:1], in_=ones_small)

            # hp with ones column (col FO) and bias pre-added; bf16 for matmul
            hp_all = singles.tile([P, NB, H, FO + 1], bf16)
            nc.vector.memset(hp_all[:, :, :, FO : FO + 1], 1.0)

            vT = singles.tile([P, NB, N], bf16)  # valid^T: [j_local, jb, i]
            identb = singles.tile([P, P], bf16)
            nc.gpsimd.tensor_copy(out=identb, in_=ident)

            # ================= phase A: prep =================
            with (
                tc.tile_pool(name="phA", bufs=1) as phA,
                tc.tile_pool(name="tempA", bufs=4) as tempA,
                tc.tile_pool(name="psumA", bufs=1, space="PSUM") as psA,
                tc.tile_pool(name="psumHP", bufs=1, space="PSUM") as psHP,
                tc.tile_pool(name="psumR", bufs=2, space="PSUM") as psR,
            ):
                w_sb = phA.tile([P, FC, H, FO], bf16)
                w_re = w_d.rearrange("h (c p) o -> p c h o", p=P)
                for fc in range(FC):
                    nc.gpsimd.dma_start(out=w_sb[:, fc], in_=w_re[:, fc])

                # a_src/a_dst: load as [16, 64], transpose to [64, (vec h)]
                a2d = phA.tile([2 * H, FO], f32)
                nc.sync.dma_start(out=a2d[0:H], in_=asrc_d[:, :])
                nc.sync.dma_start(out=a2d[H : 2 * H], in_=adst_d[:, :])
                pa2 = psR.tile([FO, 2 * H], f32, tag="hpT")
                nc.tensor.transpose(pa2, a2d, ident[0 : 2 * H, 0 : 2 * H])
                a2T = phA.tile([FO, 2 * H], bf16)
                nc.vector.tensor_copy(out=a2T, in_=pa2)
                bias_b = phA.tile([P, FO], f32)
                nc.sync.dma_start(
                    out=bias_b,
                    in_=bass.AP(tensor=bias_d, offset=0, ap=[[0, P], [1, FO]]),
                )

                h_sb = phA.tile([P, NB, FI], bf16)
                h_re = h_d.rearrange("(nb p) f -> p nb f", p=P)
                for nb in range(NB):
                    nc.gpsimd.dma_start(out=h_sb[:, nb], in_=h_re[:, nb])

                for nb in range(NB):
                    for fc in range(FC):
                        pt = psA.tile([P, P], bf16, tag="tr")
                        nc.tensor.transpose(
                            pt, h_sb[:, nb, fc * P : (fc + 1) * P], identb
                        )
                        nc.scalar.copy(
                            out=hT_sb[:, fc, nb * P : (nb + 1) * P], in_=pt
                        )

                # ----- src/dst rows via transposed h_prime (per head) -----
                # hpT[o, n] = sum_f w[f, o] h[n, f]; tanhT; then a2T.T @ tanhT
                # gives (src; dst) rows directly in row layout.
                sdrows = phA.tile([2, H, N], bf16)
                for hd in range(H):
                    phT = psR.tile([FO, N], f32, tag="hpT")
                    for fc in range(FC):
                        for half in range(2):
                            nc.tensor.matmul(
                                phT[:, half * 512 : (half + 1) * 512],
                                lhsT=w_sb[:, fc, hd, :],
                                rhs=hT_sb[:, fc, half * 512 : (half + 1) * 512],
                                start=(fc == 0),
                                stop=(fc == FC - 1),
                                skip_group_check=True,
                            )
                    tT = tempA.tile([FO, N], bf16, tag="tT")
                    nc.scalar.activation(out=tT, in_=phT, func=AF.Tanh)
                    a2_ap = bass.AP(
                        tensor=a2T.tensor,
                        offset=a2T.offset + hd,
                        ap=[list(a2T.ap[0]), [H, 2]],
                    )
                    psd = psHP.tile([2, N], f32, tag="sd")
                    for half in range(2):
                        nc.tensor.matmul(
                            psd[:, half * 512 : (half + 1) * 512],
                            lhsT=a2_ap,
                            rhs=tT[:, half * 512 : (half + 1) * 512],
                            start=True,
                            stop=True,
                        )
                    nc.vector.tensor_copy(out=sdrows[:, hd, :], in_=psd)

                # row 0 = srcrow per head; row 1 = dstrow per head
                nc.sync.dma_start(out=os2[1:2], in_=sdrows[0:1])
                nc.sync.dma_start(out=sd2[0:1], in_=sdrows[1:2])

                # ----- hp (+bias, bf16) for the output matmul -----
                for nb in range(NB):
                    php = psHP.tile([P, H, FO], f32, tag="hp")
                    for fc in range(FC):
                        nc.tensor.matmul(
                            php,
                            lhsT=hT_sb[:, fc, nb * P : (nb + 1) * P],
                            rhs=w_sb[:, fc],
                            start=(fc == 0),
                            stop=(fc == FC - 1),
                        )
                    bias_rep = bass.AP(
                        tensor=bias_b.tensor,
                        offset=bias_b.offset,
                        ap=[list(bias_b.ap[0]), [0, H], list(bias_b.ap[1])],
                    )
                    nc.vector.tensor_add(
                        out=hp_all[:, nb, :, 0:FO], in0=php, in1=bias_rep
                    )

                # ----- adjacency: valid^T via DMA transpose, bf16 -----
                for ib in range(NB):
                    adj_u8 = tempA.tile([P, N], u8, tag="adj8")
                    nc.sync.dma_start(
                        out=adj_u8, in_=adj_d[ib * P : (ib + 1) * P, :]
                    )
                    adjb = tempA.tile([P, N], bf16, tag="adjb")
                    nc.gpsimd.tensor_copy(out=adjb, in_=adj_u8)
                    nc.sync.dma_start_transpose(
                        out=vT[:, :, ib * P : (ib + 1) * P], in_=adjb
                    )
                # diagonal always valid: OR identity onto diag blocks
                for jb in range(NB):
                    nc.vector.tensor_max(
                        out=vT[:, jb, jb * P : (jb + 1) * P],
                        in0=vT[:, jb, jb * P : (jb + 1) * P],
                        in1=identb,
                    )

            # ================= phase B: attention =================
            with (
                tc.tile_pool(name="psumS", bufs=3, space="PSUM") as psS,
                tc.tile_pool(name="psumO", bufs=1, space="PSUM") as psO,
            ):
                for hd in range(H):
                    s_sb = spool.tile([P, NB, N], bf16, tag="s")
                    for jb in range(NB):
                        ps = psS.tile([P, N], f32, tag="spre")
                        for half in range(2):
                            nc.tensor.matmul(
                                ps[:, half * 512 : (half + 1) * 512],
                                lhsT=sd2[:, hd, jb * P : (jb + 1) * P],
                                rhs=os2[:, hd, half * 512 : (half + 1) * 512],
                                start=True,
                                stop=True,
                            )
                        if jb >= 3:
                            nc.scalar.activation(
                                out=s_sb[:, jb, :], in_=ps, func=AF.Prelu,
                                alpha=ALPHA,
                            )
                        else:
                            # DVE leaky: s = 0.2*ps (bf16), then s = max(5*s, s)
                            nc.vector.tensor_scalar_mul(
                                out=s_sb[:, jb, :], in0=ps, scalar1=ALPHA
                            )
                            nc.vector.scalar_tensor_tensor(
                                out=s_sb[:, jb, :], in0=s_sb[:, jb, :],
                                scalar=5.0, in1=s_sb[:, jb, :],
                                op0=OP.mult, op1=OP.max,
                            )
                    p_sb = ppool.tile([P, NB, N], bf16, tag="p")
                    nc.scalar.activation(
                        out=p_sb[:, 0:4], in_=s_sb[:, 0:4], func=AF.Exp
                    )
                    nc.scalar.activation(
                        out=p_sb[:, 4:8], in_=s_sb[:, 4:8], func=AF.Exp
                    )
                    nc.vector.tensor_mul(out=p_sb, in0=p_sb, in1=vT)

                    po_a = psO.tile([P, 4, FO + 1], f32, tag="o2a")
                    po_b = psO.tile([P, 4, FO + 1], f32, tag="o2b")
                    for ic in range(NB):
                        po = po_a if ic < 4 else po_b
                        icl = ic % 4
                        for jb in range(NB):
                            lhsT_str = bass.AP(
                                tensor=p_sb.tensor,
                                offset=p_sb[:, jb, ic : ic + 1].offset,
                                ap=[list(p_sb.ap[0]), [NB, P]],
                            )
                            nc.tensor.matmul(
                                po[:, icl, :],
                                lhsT=lhsT_str,
                                rhs=hp_all[:, jb, hd, :],
                                start=(jb == 0),
                                stop=(jb == NB - 1),
                            )

                    rz = temps.tile([P, NB, 1], f32, tag="rz")
                    nc.vector.reciprocal(out=rz[:, 0:4], in_=po_a[:, :, FO : FO + 1])
                    nc.vector.reciprocal(out=rz[:, 4:8], in_=po_b[:, :, FO : FO + 1])
                    o_sb = temps.tile([P, NB, FO], f32, tag="osb")
                    rzb_a = bass.AP(
                        tensor=rz.tensor, offset=rz[:, 0:4, :].offset,
                        ap=[list(rz.ap[0]), [rz.ap[1][0], 4], [0, FO]],
                    )
                    rzb_b = bass.AP(
                        tensor=rz.tensor, offset=rz[:, 4:8, :].offset,
                        ap=[list(rz.ap[0]), [rz.ap[1][0], 4], [0, FO]],
                    )
                    nc.vector.tensor_mul(
                        out=o_sb[:, 0:4, :], in0=po_a[:, :, 0:FO], in1=rzb_a
                    )
                    nc.vector.tensor_mul(
                        out=o_sb[:, 4:8, :], in0=po_b[:, :, 0:FO], in1=rzb_b
                    )
                    out_eng = nc.gpsimd if hd % 2 == 0 else nc.sync
                    out_eng.dma_start(
                        out=out_d[hd].rearrange("(p ic) o -> p ic o", ic=NB),
                        in_=o_sb,
                    )
    nc.finalize()
    return nc


_NC_CACHE = None
TRACE = False
LAST_RESULT = None


def kernel(h, adj, w, a_src, a_dst, bias):
    global _NC_CACHE
    from concourse.bass_utils import run_bass_kernel_spmd

    if _NC_CACHE is None:
        _NC_CACHE = build_bass()
    nc = _NC_CACHE

    h = np.ascontiguousarray(np.asarray(h, dtype=np.float32))
    adj_u8 = np.ascontiguousarray(np.asarray(adj).astype(np.uint8))
    w = np.ascontiguousarray(np.asarray(w, dtype=np.float32))
    a_src2 = np.ascontiguousarray(np.asarray(a_src, dtype=np.float32)[..., 0])
    a_dst2 = np.ascontiguousarray(np.asarray(a_dst, dtype=np.float32)[..., 0])
    bias = np.ascontiguousarray(np.asarray(bias, dtype=np.float32))

    in_maps = [
        {
            "h": h[b],
            "adj": adj_u8[b],
            "w": w,
            "a_src": a_src2,
            "a_dst": a_dst2,
            "bias": bias,
        }
        for b in range(BS)
    ]
    res = run_bass_kernel_spmd(
        nc, in_maps, core_ids=list(range(BS)), trace=TRACE,
        trace_cores=list(range(BS)) if TRACE else None,
    )
    if TRACE:
        global LAST_RESULT
        LAST_RESULT = res
    out = np.stack([r["out"] for r in res.results], axis=0)
    return out.astype(np.float32)

